# revision 1
# baseline (speedup 1.0000x reference)
"""MoE (8 experts, top-2) Trainium2 kernel.

Strategy: expert-parallel across the 8 NeuronCores. The tiny gate matmul +
top-k routing runs on host (it is the sharding step: tokens are dispatched
to the core that owns their expert). Each core runs a dense 2-layer FFN
(bf16 matmuls, fp32 accumulation) over its gathered tokens, everything in
transposed layout so biases are per-partition and no on-device transposes
are needed:

    h^T = relu(W1^T-chunks @ x^T + b1)   [F on partitions, tokens free]
    y^T = W2^T-chunks @ h^T + b2         [D on partitions, tokens free]

Host then scales by the softmax gates and scatter-adds the two expert
contributions per token.
"""

import numpy as np
import ml_dtypes

D_MODEL = 1024
D_FF = 4096
N_EXPERTS = 8
CAP = 2560          # per-expert token capacity (actual max for the fixed inputs ~2182)
TOK = 512           # tokens per device tile (matmul free dim; one PSUM bank)
NT = CAP // TOK
P = 128
KD = D_MODEL // P   # 8 contraction chunks for layer 1 / output chunks for layer 2
KF = D_FF // P      # 32 f-chunks

BF16 = ml_dtypes.bfloat16

_compiled_nc = None


def _build_bass():
    import concourse.bacc as bacc
    import concourse.mybir as mybir
    import concourse.tile as tile

    dt = mybir.dt
    AF = mybir.ActivationFunctionType

    nc = bacc.Bacc("TRN2", target_bir_lowering=False, debug=False)

    xT = nc.dram_tensor("xT", [D_MODEL, CAP], dt.bfloat16, kind="ExternalInput")
    w1 = nc.dram_tensor("w1", [D_MODEL, D_FF], dt.bfloat16, kind="ExternalInput")
    w2 = nc.dram_tensor("w2", [D_FF, D_MODEL], dt.bfloat16, kind="ExternalInput")
    b1 = nc.dram_tensor("b1", [D_FF], dt.float32, kind="ExternalInput")
    b2 = nc.dram_tensor("b2", [D_MODEL], dt.float32, kind="ExternalInput")
    yT = nc.dram_tensor("yT", [D_MODEL, CAP], dt.float32, kind="ExternalOutput")

    with tile.TileContext(nc) as tc:
        with (
            tc.tile_pool(name="wpool", bufs=1) as wpool,
            tc.tile_pool(name="hpool", bufs=1) as hpool,
            tc.tile_pool(name="xpool", bufs=2) as xpool,
            tc.tile_pool(name="ypool", bufs=4) as ypool,
            tc.tile_pool(name="bpool", bufs=1) as bpool,
            tc.tile_pool(name="ps1", bufs=3, space="PSUM") as ps1,
            tc.tile_pool(name="ps2", bufs=3, space="PSUM") as ps2,
        ):
            w1_sb = wpool.tile([P, KD, D_FF], dt.bfloat16, tag="w1")
            w2_sb = wpool.tile([P, KF, D_MODEL], dt.bfloat16, tag="w2")
            for k in range(KD):
                nc.sync.dma_start(w1_sb[:, k, :], w1[k * P:(k + 1) * P, :])
            for k in range(KF):
                nc.sync.dma_start(w2_sb[:, k, :], w2[k * P:(k + 1) * P, :])

            b1_sb = bpool.tile([P, KF], dt.float32, tag="b1")
            b2_sb = bpool.tile([P, KD], dt.float32, tag="b2")
            nc.sync.dma_start(b1_sb[:], b1.rearrange("(f p) -> p f", p=P))
            nc.sync.dma_start(b2_sb[:], b2.rearrange("(d p) -> p d", p=P))

            for t in range(NT):
                lo = t * TOK
                hi = lo + TOK
                x_sb = xpool.tile([P, KD, TOK], dt.bfloat16, tag="x")
                for k in range(KD):
                    nc.sync.dma_start(x_sb[:, k, :], xT[k * P:(k + 1) * P, lo:hi])

                h_sb = hpool.tile([P, KF, TOK], dt.bfloat16, tag="h")
                for f in range(KF):
                    ph = ps1.tile([P, TOK], dt.float32, tag="ph")
                    for k in range(KD):
                        nc.tensor.matmul(
                            ph[:],
                            w1_sb[:, k, f * P:(f + 1) * P],
                            x_sb[:, k, :],
                            start=(k == 0),
                            stop=(k == KD - 1),
                        )
                    nc.scalar.activation(
                        h_sb[:, f, :], ph[:], AF.Relu, bias=b1_sb[:, f:f + 1], scale=1.0
                    )

                for d in range(KD):
                    py = ps2.tile([P, TOK], dt.float32, tag="py")
                    for kf in range(KF):
                        nc.tensor.matmul(
                            py[:],
                            w2_sb[:, kf, d * P:(d + 1) * P],
                            h_sb[:, kf, :],
                            start=(kf == 0),
                            stop=(kf == KF - 1),
                        )
                    y_sb = ypool.tile([P, TOK], dt.float32, tag="y")
                    nc.scalar.activation(
                        y_sb[:], py[:], AF.Identity, bias=b2_sb[:, d:d + 1], scale=1.0
                    )
                    nc.sync.dma_start(yT[d * P:(d + 1) * P, lo:hi], y_sb[:])

    nc.compile()
    return nc


def _get_nc():
    global _compiled_nc
    if _compiled_nc is None:
        _compiled_nc = _build_bass()
    return _compiled_nc


def _route(x, Wg, bg, k):
    """Host gating: returns (idx_list, gate_list) per expert."""
    logits = x.astype(np.float64) @ Wg.astype(np.float64) + bg.astype(np.float64)
    n = logits.shape[0]
    # top-k indices (order within the k does not matter: the weighted sum is
    # permutation invariant)
    topk = np.argpartition(-logits, k - 1, axis=1)[:, :k]
    vals = np.take_along_axis(logits, topk, axis=1)
    vals = vals - vals.max(axis=1, keepdims=True)
    ev = np.exp(vals)
    gates = (ev / ev.sum(axis=1, keepdims=True)).astype(np.float32)

    idx_list, gate_list = [], []
    for e in range(N_EXPERTS):
        rows, cols = np.nonzero(topk == e)
        idx_list.append(rows.astype(np.int64))
        gate_list.append(gates[rows, cols])
    return idx_list, gate_list


def _ffn_host(xs, W1e, b1e, W2e, b2e):
    """Overflow fallback: exact fp32 FFN on host for a few tokens."""
    h = np.maximum(xs @ W1e + b1e, 0.0)
    return h @ W2e + b2e


def kernel(x, Wg, bg, W1, b1, W2, b2, k, _run_opts=None):
    from concourse.bass_utils import run_bass_kernel_spmd

    x = np.asarray(x, dtype=np.float32)
    Wg = np.asarray(Wg, dtype=np.float32)
    bg = np.asarray(bg, dtype=np.float32)
    W1 = np.asarray(W1, dtype=np.float32)
    b1 = np.asarray(b1, dtype=np.float32)
    W2 = np.asarray(W2, dtype=np.float32)
    b2 = np.asarray(b2, dtype=np.float32)
    k = int(k)

    n_tokens = x.shape[0]
    idx_list, gate_list = _route(x, Wg, bg, k)

    xT_bf = np.ascontiguousarray(x.T).astype(BF16)  # [D, N]

    in_maps = []
    for e in range(N_EXPERTS):
        idx = idx_list[e][:CAP]
        xg = np.zeros((D_MODEL, CAP), dtype=BF16)
        xg[:, :len(idx)] = xT_bf[:, idx]
        in_maps.append({
            "xT": xg,
            "w1": W1[e].astype(BF16),
            "w2": W2[e].astype(BF16),
            "b1": b1[e],
            "b2": b2[e],
        })

    nc = _get_nc()
    res = run_bass_kernel_spmd(
        nc, in_maps, core_ids=list(range(N_EXPERTS)), **(_run_opts or {})
    )

    out = np.zeros((n_tokens, D_MODEL), dtype=np.float32)
    for e in range(N_EXPERTS):
        idx = idx_list[e]
        g = gate_list[e]
        n_e = min(len(idx), CAP)
        ye = res.results[e]["yT"][:, :n_e].T  # [n_e, D]
        out[idx[:n_e]] += g[:n_e, None] * ye
        if len(idx) > CAP:  # overflow fallback (cannot happen for the fixed inputs)
            extra = idx[CAP:]
            ye_extra = _ffn_host(x[extra], W1[e], b1[e], W2[e], b2[e])
            out[extra] += g[CAP:, None] * ye_extra

    if _run_opts:
        kernel._last_results = res
    return out
